# revision 7
# baseline (speedup 1.0000x reference)
"""Trainium2 Bass kernel for nn_GB_GLHF_1288490189083.

Data-parallel over batch: each of the 8 NeuronCores processes one population
(b=1 slice). All model weights are replicated; batchPop / mut_rand /
cross_rand / gumbel_u are sharded on dim 0.

Per-core pipeline:
  comparison-count ranks + indirect-DMA scatter sort -> fitness token
  -> tiny MLP -> q/k projections (fp32r matmuls) -> A = tanh(q k^T/sqrt(QK))
  masked, and its transpose via swapped matmul operands
  -> vchrom = A @ chrom (fp32r) -> cosine-sim token -> crossover-rate MLP
  -> gumbel-softmax hard selection -> offspring fitness + 1-to-1 selection.
"""
import sys
if '/opt/trn_rl_repo' not in sys.path:
    sys.path.insert(0, '/opt/trn_rl_repo')
import numpy as np

B, N, D = 8, 1000, 2000
H2, QK, CRH = 100, 1000, 100
NT = 8
TSZ = [128] * 7 + [104]
TST = [128 * t for t in range(NT)]
N_CORES = 8

_cache = {}


def _build():
    import concourse.bacc as bacc
    import concourse.bass as bass
    import concourse.mybir as mybir
    import concourse.tile as tile
    from concourse.tile import add_dep_helper as _adh
    from concourse.masks import make_identity

    def add_dep_helper(a, b, reason=""):
        ai = a.ins if hasattr(a, "ins") and not isinstance(a.ins, list) else a
        bi = b.ins if hasattr(b, "ins") and not isinstance(b.ins, list) else b
        _adh(ai, bi, reason=reason)

    AF = mybir.ActivationFunctionType
    ALU = mybir.AluOpType
    F32 = mybir.dt.float32
    F32R = mybir.dt.float32r
    BF16 = mybir.dt.bfloat16
    I32 = mybir.dt.int32
    U8 = mybir.dt.uint8
    AX = mybir.AxisListType.X

    nc = bacc.Bacc("TRN2", target_bir_lowering=False, debug=False,
                   num_devices=N_CORES)

    def din(name, shape):
        return nc.dram_tensor(name, shape, F32, kind="ExternalInput").ap()

    pop = din("pop", [N, D + 1])
    mutr = din("mutr", [N, N])
    crossr = din("crossr", [N, D])
    gub = din("gub", [N, 2 * D])            # gumbel_u reshaped [N, D*2]
    wwT = din("wwT", [2, H2])
    wb = din("wb", [H2])
    g1 = din("g1", [H2])
    b1 = din("b1", [H2])
    fqwT = din("fqwT", [H2, QK])
    fqb = din("fqb", [QK])
    fkwT = din("fkwT", [H2, QK])
    fkb = din("fkb", [QK])
    crw1T = din("crw1T", [3, CRH])
    crb1 = din("crb1", [CRH])
    crg = din("crg", [CRH])
    crb = din("crb", [CRH])
    crw2 = din("crw2", [CRH])
    crb2 = din("crb2", [1])
    rtok = din("rtok", [N])

    onext = nc.dram_tensor("onext", [N, D + 1], F32, kind="ExternalOutput").ap()
    oA = nc.dram_tensor("oA", [N, N], F32, kind="ExternalOutput").ap()
    ocr = nc.dram_tensor("ocr", [N], F32, kind="ExternalOutput").ap()

    spopd = nc.dram_tensor("spopd", [N, D + 1], F32, kind="Internal").ap()
    vchd = nc.dram_tensor("vchd", [N, D], F32, kind="Internal").ap()
    rnkd = nc.dram_tensor("rnkd", [N], I32, kind="Internal").ap()
    simd_ = nc.dram_tensor("simd_", [N], F32, kind="Internal").ap()

    ASCL = float(1.0 / np.sqrt(np.float32(QK), dtype=np.float32))

    with nc.allow_non_contiguous_dma(reason="small column loads/stores"), \
         tile.TileContext(nc) as tc:
        with tc.tile_pool(name="G", bufs=1) as G:
            # ---------------- constants / weights ----------------
            ident = G.tile([128, 128], F32, name="ident")
            make_identity(nc, ident[:])
            eyebf = G.tile([128, 128], BF16, name="eyebf")
            nc.vector.tensor_copy(eyebf[:], ident[:])
            onesbf = G.tile([128, 1], BF16, name="onesbf")
            nc.vector.memset(onesbf[:], 1.0)
            eps10 = G.tile([128, 1], F32, name="eps10")
            nc.vector.memset(eps10[:], 1e-10)
            epsLN = G.tile([128, 1], F32, name="epsLN")
            nc.vector.memset(epsLN[:], 1e-5)

            wwT_s = G.tile([2, H2], F32, name="wwT_s")
            nc.sync.dma_start(wwT_s[:], wwT)
            crw1T_s = G.tile([3, CRH], F32, name="crw1T_s")
            nc.sync.dma_start(crw1T_s[:], crw1T)

            def brow(name, src, width):
                row = G.tile([1, width], F32, name=name + "_r")
                nc.sync.dma_start(row[:], src.rearrange("(o f) -> o f", o=1))
                rep = G.tile([128, width], F32, name=name + "_rep")
                nc.gpsimd.partition_broadcast(rep[:], row[:1, :])
                return rep

            wb_rep = brow("wb", wb, H2)
            g1_rep = brow("g1", g1, H2)
            b1_rep = brow("b1", b1, H2)
            crb1_rep = brow("crb1", crb1, CRH)
            crg_rep = brow("crg", crg, CRH)
            crb_rep = brow("crb", crb, CRH)
            w2_rep = brow("w2", crw2, CRH)
            crb2_rep = brow("crb2", crb2, 1)

            def col(name, src, t):
                c = G.tile([TSZ[t], 1], F32, name=name)
                nc.sync.dma_start(
                    c[:], src[TST[t]:TST[t] + TSZ[t]].rearrange("(p o) -> p o", o=1))
                return c

            fqb_c = [col(f"fqb{t}", fqb, t) for t in range(NT)]
            fkb_c = [col(f"fkb{t}", fkb, t) for t in range(NT)]

            # ---------------- S1: ranks ----------------
            fitc = []
            for t in range(NT):
                c = G.tile([TSZ[t], 1], F32, name=f"fitc{t}")
                nc.sync.dma_start(c[:], pop[TST[t]:TST[t] + TSZ[t], 0:1])
                fitc.append(c)
            fitrow = G.tile([1, N], F32, name="fitrow")
            nc.sync.dma_start(fitrow[:], pop[:, 0:1].rearrange("n o -> o n"))
            fitrep = G.tile([128, N], F32, name="fitrep")
            nc.gpsimd.partition_broadcast(fitrep[:], fitrow[:1, :])

            # unsorted mean/std1 of fit (permutation-invariant)
            fbn = G.tile([1, 12], F32, name="fbn")
            for c_ in range(2):
                nc.vector.bn_stats(fbn[:, 6 * c_:6 * (c_ + 1)],
                                   fitrow[:, 500 * c_:500 * (c_ + 1)])
            fst = G.tile([1, 2], F32, name="fst")
            nc.vector.bn_aggr(fst[:], fbn[:].rearrange("o (c s) -> o c s", s=6))
            fvar1 = G.tile([1, 1], F32, name="fvar1")
            nc.vector.tensor_scalar(out=fvar1[:], in0=fst[:, 1:2],
                                    scalar1=float(N) / float(N - 1), scalar2=None,
                                    op0=ALU.mult)
            fsd = G.tile([1, 1], F32, name="fsd")
            nc.scalar.activation(fsd[:], fvar1[:], AF.Sqrt)
            finv = G.tile([1, 1], F32, name="finv")
            nc.vector.reciprocal(finv[:], fsd[:])

            ri32 = G.tile([1, N], I32, name="ri32")
            with tc.tile_pool(name="RNK", bufs=1) as RNK, \
                 tc.tile_pool(name="PS0", bufs=1, space="PSUM") as PS0:
                ltc, eqm = [], []
                for t in range(NT):
                    P = TSZ[t]
                    lt = RNK.tile([P, N], BF16, name=f"lt{t}")
                    nc.vector.tensor_scalar(out=lt[:], in0=fitrep[:P, :],
                                            scalar1=fitc[t][:, 0:1], scalar2=None,
                                            op0=ALU.is_gt)
                    eq = RNK.tile([P, N], BF16, name=f"eq{t}")
                    nc.vector.tensor_scalar(out=eq[:], in0=fitrep[:P, :],
                                            scalar1=fitc[t][:, 0:1], scalar2=None,
                                            op0=ALU.is_equal)
                    em = RNK.tile([P, N], BF16, name=f"em{t}")
                    nc.gpsimd.affine_select(em[:], eq[:], pattern=[[1, N]],
                                            compare_op=ALU.is_gt, fill=0.0,
                                            base=-TST[t], channel_multiplier=-1)
                    ltc.append(lt)
                    eqm.append(em)
                for ch in range(2):
                    sl = slice(500 * ch, 500 * (ch + 1))
                    ps = PS0.tile([1, 500], F32, name=f"rps{ch}", tag=f"rps{ch}")
                    for t in range(NT):
                        P = TSZ[t]
                        nc.tensor.matmul(ps[:], onesbf[:P, :], ltc[t][:, sl],
                                         start=(t == 0), stop=False)
                        nc.tensor.matmul(ps[:], onesbf[:P, :], eqm[t][:, sl],
                                         start=False, stop=(t == NT - 1))
                    nc.vector.tensor_copy(ri32[:, sl], ps[:])
            w_rnk = nc.sync.dma_start(rnkd.rearrange("(o f) -> o f", o=1), ri32[:])
            rankc = []
            for t in range(NT):
                rc = G.tile([TSZ[t], 1], I32, name=f"rankc{t}")
                i = nc.sync.dma_start(
                    rc[:], rnkd[TST[t]:TST[t] + TSZ[t]].rearrange("(p o) -> p o", o=1))
                add_dep_helper(i, w_rnk, reason="rank cols after rank row write")
                rankc.append(rc)

            # ---------------- S2: scatter rows into sorted order ----------------
            scat_insts = []
            with tc.tile_pool(name="SCT", bufs=1) as SCT:
                for t in range(NT):
                    P = TSZ[t]
                    pt = SCT.tile([128, D + 1], F32, name=f"popt{t}", tag="popt", bufs=2)
                    nc.sync.dma_start(pt[:P, :], pop[TST[t]:TST[t] + TSZ[t], :])
                    si = nc.gpsimd.indirect_dma_start(
                        out=spopd,
                        out_offset=bass.IndirectOffsetOnAxis(ap=rankc[t][:, 0:1], axis=0),
                        in_=pt[:P, :], in_offset=None)
                    scat_insts.append(si)

            # ---------------- S3: sorted fit + fn token ----------------
            sfitc = []
            for t in range(NT):
                c = G.tile([TSZ[t], 1], F32, name=f"sfitc{t}")
                i = nc.sync.dma_start(c[:], spopd[TST[t]:TST[t] + TSZ[t], 0:1])
                for si in scat_insts:
                    add_dep_helper(i, si, reason="sorted fit after scatter")
                sfitc.append(c)
            sfitrow = G.tile([1, N], F32, name="sfitrow")
            i = nc.sync.dma_start(sfitrow[:], spopd[:, 0:1].rearrange("n o -> o n"))
            for si in scat_insts:
                add_dep_helper(i, si, reason="sorted fit row after scatter")

            token3T = G.tile([3, N], F32, name="token3T")
            nc.vector.tensor_scalar(out=token3T[0:1, :], in0=sfitrow[:],
                                    scalar1=fst[:, 0:1], scalar2=finv[:, 0:1],
                                    op0=ALU.subtract, op1=ALU.mult)
            nc.sync.dma_start(token3T[1:2, :], rtok.rearrange("(o f) -> o f", o=1))

            simc = [G.tile([TSZ[t], 1], F32, name=f"simc{t}") for t in range(NT)]
            crc = [G.tile([TSZ[t], 1], F32, name=f"crc{t}") for t in range(NT)]
            vchd_w = []

            with tc.tile_pool(name="ATP", bufs=1) as ATP:
                ATr = [ATP.tile([TSZ[t], N], F32R, name=f"ATr{t}") for t in range(NT)]
                with tc.tile_pool(name="QKP", bufs=1) as QKP:
                    # ---------------- S4: h layer + hT ----------------
                    hT = QKP.tile([H2, N], F32R, name="hT")
                    with tc.tile_pool(name="MLP", bufs=1) as MLP, \
                         tc.tile_pool(name="PSM", bufs=1, space="PSUM") as PSM:
                        for t in range(NT):
                            P = TSZ[t]
                            hp = PSM.tile([P, H2], F32, name=f"hp{t}", tag="hp", bufs=2)
                            nc.tensor.matmul(hp[:], token3T[0:2, TST[t]:TST[t] + P],
                                             wwT_s[:], start=True, stop=True)
                            hb = MLP.tile([128, H2], F32, name=f"hb{t}", tag="hb", bufs=2)
                            nc.vector.tensor_tensor(out=hb[:P, :], in0=hp[:],
                                                    in1=wb_rep[:P, :], op=ALU.add)
                            nc.vector.tensor_scalar(out=hb[:P, :], in0=hb[:P, :],
                                                    scalar1=0.0, scalar2=None,
                                                    op0=ALU.max)
                            bn6 = MLP.tile([128, 6], F32, name=f"bn6{t}", tag="bn6",
                                           bufs=2)
                            nc.vector.bn_stats(bn6[:P, :], hb[:P, :])
                            bn2 = MLP.tile([128, 2], F32, name=f"bn2{t}", tag="bn2",
                                           bufs=2)
                            nc.vector.bn_aggr(bn2[:P, :], bn6[:P, :].rearrange(
                                "p (c s) -> p c s", c=1))
                            sd = MLP.tile([128, 1], F32, name=f"sd{t}", tag="sd", bufs=2)
                            nc.scalar.activation(sd[:P, :], bn2[:P, 1:2], AF.Sqrt,
                                                 bias=epsLN[:P, :])
                            iv = MLP.tile([128, 1], F32, name=f"iv{t}", tag="iv", bufs=2)
                            nc.vector.reciprocal(iv[:P, :], sd[:P, :])
                            nc.vector.tensor_scalar(out=hb[:P, :], in0=hb[:P, :],
                                                    scalar1=bn2[:P, 0:1],
                                                    scalar2=iv[:P, 0:1],
                                                    op0=ALU.subtract, op1=ALU.mult)
                            nc.vector.tensor_tensor(out=hb[:P, :], in0=hb[:P, :],
                                                    in1=g1_rep[:P, :], op=ALU.mult)
                            nc.vector.tensor_tensor(out=hb[:P, :], in0=hb[:P, :],
                                                    in1=b1_rep[:P, :], op=ALU.add)
                            tp = PSM.tile([H2, P], F32, name=f"tp{t}", tag="tp", bufs=2)
                            nc.tensor.transpose(tp[:], hb[:P, :H2], ident[:P, :P])
                            nc.vector.tensor_copy(hT[:, TST[t]:TST[t] + P], tp[:])

                    # ---------------- S5: q1T / k1T ----------------
                    fqwT_s = QKP.tile([H2, QK], F32R, name="fqwT_s")
                    nc.sync.dma_start(fqwT_s[:], fqwT.bitcast(F32R))
                    fkwT_s = QKP.tile([H2, QK], F32R, name="fkwT_s")
                    nc.sync.dma_start(fkwT_s[:], fkwT.bitcast(F32R))
                    q1T = [QKP.tile([TSZ[t], QK], F32R, name=f"q1T{t}")
                           for t in range(NT)]
                    k1T = [QKP.tile([TSZ[t], QK], F32R, name=f"k1T{t}")
                           for t in range(NT)]
                    with tc.tile_pool(name="PSQ", bufs=1, space="PSUM") as PSQ:
                        for mt in range(NT):
                            P = TSZ[mt]
                            for ch in range(2):
                                sl = slice(500 * ch, 500 * (ch + 1))
                                pq = PSQ.tile([P, 500], F32, name=f"pq{mt}{ch}",
                                              tag="pq", bufs=2)
                                nc.tensor.matmul(pq[:], fqwT_s[:, TST[mt]:TST[mt] + P],
                                                 hT[:, sl], start=True, stop=True)
                                nc.scalar.activation(q1T[mt][:, sl], pq[:], AF.Tanh,
                                                     bias=fqb_c[mt][:, 0:1])
                                pk = PSQ.tile([P, 500], F32, name=f"pk{mt}{ch}",
                                              tag="pk", bufs=2)
                                nc.tensor.matmul(pk[:], fkwT_s[:, TST[mt]:TST[mt] + P],
                                                 hT[:, sl], start=True, stop=True)
                                nc.scalar.activation(k1T[mt][:, sl], pk[:], AF.Tanh,
                                                     bias=fkb_c[mt][:, 0:1])

                    # ---------------- S6: A, AT, masks ----------------
                    with tc.tile_pool(name="MSKP", bufs=1) as MSKP, \
                         tc.tile_pool(name="PSA", bufs=1, space="PSUM") as PSA:
                        maskE = [MSKP.tile([128, 1024], BF16, name=f"maskE{t}")
                                 for t in range(NT)]
                        maskT = [MSKP.tile([128, 1024], BF16, name=f"maskT{t}")
                                 for t in range(NT)]
                        for t in range(NT):
                            P = TSZ[t]
                            nc.vector.memset(maskE[t][:, N:], 0.0)
                            if P < 128:
                                nc.vector.memset(maskE[t][96:, :N], 0.0)
                            mu_ = MSKP.tile([128, N], F32, name=f"mut{t}", tag="mut",
                                            bufs=2)
                            nc.sync.dma_start(mu_[:P, :], mutr[TST[t]:TST[t] + P, :])
                            nc.vector.tensor_scalar(out=maskE[t][:P, :N], in0=mu_[:P, :],
                                                    scalar1=0.5, scalar2=None,
                                                    op0=ALU.is_ge)
                            nc.vector.tensor_tensor(
                                out=maskE[t][:P, TST[t]:TST[t] + P],
                                in0=maskE[t][:P, TST[t]:TST[t] + P],
                                in1=eyebf[:P, :P], op=ALU.max)
                        # A (natural) -> masked -> DMA out
                        for i in range(NT):
                            P = TSZ[i]
                            at_ = MSKP.tile([128, N], F32, name=f"at{i}", tag="at",
                                            bufs=2)
                            for ch in range(2):
                                sl = slice(500 * ch, 500 * (ch + 1))
                                pa = PSA.tile([P, 500], F32, name=f"pa{i}{ch}",
                                              tag="pa", bufs=2)
                                for mt in range(NT):
                                    nc.tensor.matmul(pa[:],
                                                     q1T[mt][:, TST[i]:TST[i] + P],
                                                     k1T[mt][:, sl], start=(mt == 0),
                                                     stop=(mt == NT - 1))
                                nc.scalar.activation(at_[:P, sl], pa[:], AF.Tanh,
                                                     scale=ASCL)
                            nc.vector.tensor_tensor(out=at_[:P, :N], in0=at_[:P, :N],
                                                    in1=maskE[i][:P, :N], op=ALU.mult)
                            nc.sync.dma_start(oA[TST[i]:TST[i] + P, :], at_[:P, :N])
                        # mask transpose blocks (bf16 xbar DMA transpose)
                        for t in range(NT):
                            for j in range(NT):
                                nc.sync.dma_start_transpose(
                                    maskT[j][:, 128 * t:128 * (t + 1)],
                                    maskE[t][:, 128 * j:128 * (j + 1)])
                        # AT (swapped operands) -> masked -> f32r
                        for i in range(NT):
                            P = TSZ[i]
                            att_ = MSKP.tile([128, N], F32, name=f"att{i}", tag="att",
                                             bufs=2)
                            for ch in range(2):
                                sl = slice(500 * ch, 500 * (ch + 1))
                                pb_ = PSA.tile([P, 500], F32, name=f"pb{i}{ch}",
                                               tag="pb", bufs=2)
                                for mt in range(NT):
                                    nc.tensor.matmul(pb_[:],
                                                     k1T[mt][:, TST[i]:TST[i] + P],
                                                     q1T[mt][:, sl], start=(mt == 0),
                                                     stop=(mt == NT - 1))
                                nc.scalar.activation(att_[:P, sl], pb_[:], AF.Tanh,
                                                     scale=ASCL)
                            nc.vector.tensor_tensor(out=ATr[i][:P, :], in0=att_[:P, :N],
                                                    in1=maskT[i][:P, :N], op=ALU.mult)

                # ---------------- S7: vchrom + sim stats ----------------
                with tc.tile_pool(name="SC", bufs=1) as SC, \
                     tc.tile_pool(name="VC", bufs=1) as VC, \
                     tc.tile_pool(name="PSV", bufs=1, space="PSUM") as PSV:
                    schrom = []
                    for t in range(NT):
                        P = TSZ[t]
                        s = SC.tile([P, D], F32R, name=f"schrom{t}")
                        i = nc.sync.dma_start(
                            s[:], spopd[TST[t]:TST[t] + P, 1:].bitcast(F32R))
                        for si in scat_insts:
                            add_dep_helper(i, si, reason="sorted chrom after scatter")
                        schrom.append(s)
                    for i in range(NT):
                        P = TSZ[i]
                        vch = VC.tile([128, D], F32, name=f"vch{i}", tag="vch", bufs=2)
                        svp = VC.tile([128, 4], F32, name=f"svp{i}", tag="svp", bufs=2)
                        for ch in range(4):
                            sl = slice(500 * ch, 500 * (ch + 1))
                            pv = PSV.tile([P, 500], F32, name=f"pv{i}{ch}", tag="pv",
                                          bufs=2)
                            for mt in range(NT):
                                nc.tensor.matmul(pv[:], ATr[mt][:, TST[i]:TST[i] + P],
                                                 schrom[mt][:, sl], start=(mt == 0),
                                                 stop=(mt == NT - 1))
                            nc.scalar.activation(vch[:P, sl], pv[:], AF.Identity,
                                                 accum_out=svp[:P, ch:ch + 1])
                        vchd_w.append(
                            nc.sync.dma_start(vchd[TST[i]:TST[i] + P, :], vch[:P, :]))
                        sv = VC.tile([128, 1], F32, name=f"sv{i}", tag="sv", bufs=2)
                        nc.vector.tensor_reduce(out=sv[:P, :], in_=svp[:P, :],
                                                op=ALU.add, axis=AX)
                        scr = VC.tile([128, D], F32, name=f"scr{i}", tag="scr", bufs=2)
                        svv = VC.tile([128, 1], F32, name=f"svv{i}", tag="svv", bufs=2)
                        nc.scalar.activation(scr[:P, :], vch[:P, :], AF.Square,
                                             accum_out=svv[:P, 0:1])
                        cbn = VC.tile([128, 24], F32, name=f"cbn{i}", tag="cbn", bufs=2)
                        for c_ in range(4):
                            nc.vector.bn_stats(
                                cbn[:P, 6 * c_:6 * (c_ + 1)],
                                schrom[i][:, 500 * c_:500 * (c_ + 1)].bitcast(F32))
                        cst = VC.tile([128, 2], F32, name=f"cst{i}", tag="cst", bufs=2)
                        nc.vector.bn_aggr(cst[:P, :],
                                          cbn[:P, :].rearrange("p (c s) -> p c s", s=6))
                        prod = VC.tile([128, D], F32, name=f"prod{i}", tag="prod",
                                       bufs=2)
                        nc.vector.tensor_tensor(out=prod[:P, :],
                                                in0=schrom[i][:].bitcast(F32),
                                                in1=vch[:P, :], op=ALU.mult)
                        scv = VC.tile([128, 1], F32, name=f"scv{i}", tag="scv", bufs=2)
                        nc.scalar.activation(scr[:P, :], prod[:P, :], AF.Identity,
                                             accum_out=scv[:P, 0:1])
                        # column math for cosine sim
                        sc_ = VC.tile([128, 1], F32, name=f"sc{i}", tag="sc_", bufs=2)
                        nc.vector.tensor_scalar(out=sc_[:P, :], in0=cst[:P, 0:1],
                                                scalar1=float(D), scalar2=None,
                                                op0=ALU.mult)
                        scc = VC.tile([128, 1], F32, name=f"scc{i}", tag="scc", bufs=2)
                        nc.vector.tensor_scalar(out=scc[:P, :], in0=cst[:P, 1:2],
                                                scalar1=float(D), scalar2=None,
                                                op0=ALU.mult)
                        msq = VC.tile([128, 1], F32, name=f"msq{i}", tag="msq", bufs=2)
                        nc.vector.tensor_scalar(out=msq[:P, :], in0=cst[:P, 0:1],
                                                scalar1=cst[:P, 0:1], scalar2=float(D),
                                                op0=ALU.mult, op1=ALU.mult)
                        nc.vector.tensor_tensor(out=scc[:P, :], in0=scc[:P, :],
                                                in1=msq[:P, :], op=ALU.add)
                        mrow = VC.tile([128, 1], F32, name=f"mrow{i}", tag="mrow",
                                       bufs=2)
                        nc.vector.tensor_tensor(out=mrow[:P, :], in0=sc_[:P, :],
                                                in1=sv[:P, :], op=ALU.add)
                        nc.vector.tensor_scalar(out=mrow[:P, :], in0=mrow[:P, :],
                                                scalar1=1.0 / (2.0 * D), scalar2=None,
                                                op0=ALU.mult)
                        m2d = VC.tile([128, 1], F32, name=f"m2d{i}", tag="m2d", bufs=2)
                        nc.vector.tensor_scalar(out=m2d[:P, :], in0=mrow[:P, :],
                                                scalar1=mrow[:P, 0:1], scalar2=float(D),
                                                op0=ALU.mult, op1=ALU.mult)
                        t1 = VC.tile([128, 1], F32, name=f"t1{i}", tag="t1", bufs=2)
                        nc.vector.tensor_scalar(out=t1[:P, :], in0=sc_[:P, :],
                                                scalar1=mrow[:P, 0:1], scalar2=-2.0,
                                                op0=ALU.mult, op1=ALU.mult)
                        nc2_ = VC.tile([128, 1], F32, name=f"nc2{i}", tag="nc2", bufs=2)
                        nc.vector.tensor_tensor(out=nc2_[:P, :], in0=scc[:P, :],
                                                in1=t1[:P, :], op=ALU.add)
                        nc.vector.tensor_tensor(out=nc2_[:P, :], in0=nc2_[:P, :],
                                                in1=m2d[:P, :], op=ALU.add)
                        t2 = VC.tile([128, 1], F32, name=f"t2{i}", tag="t2", bufs=2)
                        nc.vector.tensor_scalar(out=t2[:P, :], in0=sv[:P, :],
                                                scalar1=mrow[:P, 0:1], scalar2=-2.0,
                                                op0=ALU.mult, op1=ALU.mult)
                        nv2_ = VC.tile([128, 1], F32, name=f"nv2{i}", tag="nv2", bufs=2)
                        nc.vector.tensor_tensor(out=nv2_[:P, :], in0=svv[:P, :],
                                                in1=t2[:P, :], op=ALU.add)
                        nc.vector.tensor_tensor(out=nv2_[:P, :], in0=nv2_[:P, :],
                                                in1=m2d[:P, :], op=ALU.add)
                        dt_ = VC.tile([128, 1], F32, name=f"dt{i}", tag="dt", bufs=2)
                        nc.vector.tensor_tensor(out=dt_[:P, :], in0=sc_[:P, :],
                                                in1=sv[:P, :], op=ALU.add)
                        nc.vector.tensor_scalar(out=dt_[:P, :], in0=dt_[:P, :],
                                                scalar1=mrow[:P, 0:1], scalar2=-1.0,
                                                op0=ALU.mult, op1=ALU.mult)
                        nc.vector.tensor_tensor(out=dt_[:P, :], in0=dt_[:P, :],
                                                in1=scv[:P, :], op=ALU.add)
                        nc.vector.tensor_tensor(out=dt_[:P, :], in0=dt_[:P, :],
                                                in1=m2d[:P, :], op=ALU.add)
                        den = VC.tile([128, 1], F32, name=f"den{i}", tag="den", bufs=2)
                        nc.vector.tensor_tensor(out=den[:P, :], in0=nc2_[:P, :],
                                                in1=nv2_[:P, :], op=ALU.mult)
                        nc.scalar.activation(den[:P, :], den[:P, :], AF.Sqrt)
                        nc.vector.reciprocal(den[:P, :], den[:P, :])
                        nc.vector.tensor_tensor(out=simc[i][:, :], in0=dt_[:P, :],
                                                in1=den[:P, :], op=ALU.mult)

            # ---------------- S8: sim normalize + cr MLP ----------------
            sim_w = []
            for t in range(NT):
                sim_w.append(nc.sync.dma_start(
                    simd_[TST[t]:TST[t] + TSZ[t]].rearrange("(p o) -> p o", o=1),
                    simc[t][:, :]))
            simrow = G.tile([1, N], F32, name="simrow")
            i = nc.sync.dma_start(simrow[:], simd_.rearrange("(o f) -> o f", o=1))
            for w in sim_w:
                add_dep_helper(i, w, reason="sim row after col writes")
            sbn = G.tile([1, 12], F32, name="sbn")
            for c_ in range(2):
                nc.vector.bn_stats(sbn[:, 6 * c_:6 * (c_ + 1)],
                                   simrow[:, 500 * c_:500 * (c_ + 1)])
            sst = G.tile([1, 2], F32, name="sst")
            nc.vector.bn_aggr(sst[:], sbn[:].rearrange("o (c s) -> o c s", s=6))
            svar1 = G.tile([1, 1], F32, name="svar1")
            nc.vector.tensor_scalar(out=svar1[:], in0=sst[:, 1:2],
                                    scalar1=float(N) / float(N - 1), scalar2=None,
                                    op0=ALU.mult)
            ssd = G.tile([1, 1], F32, name="ssd")
            nc.scalar.activation(ssd[:], svar1[:], AF.Sqrt)
            sinv = G.tile([1, 1], F32, name="sinv")
            nc.vector.reciprocal(sinv[:], ssd[:])
            simn = G.tile([1, N], F32, name="simn")
            nc.vector.tensor_scalar(out=simn[:], in0=simrow[:],
                                    scalar1=sst[:, 0:1], scalar2=sinv[:, 0:1],
                                    op0=ALU.subtract, op1=ALU.mult)
            nc.sync.dma_start(token3T[2:3, :], simn[:])

            with tc.tile_pool(name="MLP2", bufs=1) as MLP2, \
                 tc.tile_pool(name="PSM2", bufs=1, space="PSUM") as PSM2:
                for t in range(NT):
                    P = TSZ[t]
                    hp = PSM2.tile([P, CRH], F32, name=f"h2p{t}", tag="h2p", bufs=2)
                    nc.tensor.matmul(hp[:], token3T[:, TST[t]:TST[t] + P],
                                     crw1T_s[:], start=True, stop=True)
                    hb = MLP2.tile([128, CRH], F32, name=f"h2b{t}", tag="h2b", bufs=2)
                    nc.vector.tensor_tensor(out=hb[:P, :], in0=hp[:],
                                            in1=crb1_rep[:P, :], op=ALU.add)
                    nc.vector.tensor_scalar(out=hb[:P, :], in0=hb[:P, :], scalar1=0.0,
                                            scalar2=None, op0=ALU.max)
                    bn6 = MLP2.tile([128, 6], F32, name=f"c6{t}", tag="c6", bufs=2)
                    nc.vector.bn_stats(bn6[:P, :], hb[:P, :])
                    bn2 = MLP2.tile([128, 2], F32, name=f"c2{t}", tag="c2", bufs=2)
                    nc.vector.bn_aggr(bn2[:P, :], bn6[:P, :].rearrange(
                        "p (c s) -> p c s", c=1))
                    sd = MLP2.tile([128, 1], F32, name=f"csd{t}", tag="csd", bufs=2)
                    nc.scalar.activation(sd[:P, :], bn2[:P, 1:2], AF.Sqrt,
                                         bias=epsLN[:P, :])
                    iv = MLP2.tile([128, 1], F32, name=f"civ{t}", tag="civ", bufs=2)
                    nc.vector.reciprocal(iv[:P, :], sd[:P, :])
                    nc.vector.tensor_scalar(out=hb[:P, :], in0=hb[:P, :],
                                            scalar1=bn2[:P, 0:1], scalar2=iv[:P, 0:1],
                                            op0=ALU.subtract, op1=ALU.mult)
                    nc.vector.tensor_tensor(out=hb[:P, :], in0=hb[:P, :],
                                            in1=crg_rep[:P, :], op=ALU.mult)
                    nc.vector.tensor_tensor(out=hb[:P, :], in0=hb[:P, :],
                                            in1=crb_rep[:P, :], op=ALU.add)
                    nc.vector.tensor_tensor(out=hb[:P, :], in0=hb[:P, :],
                                            in1=w2_rep[:P, :], op=ALU.mult)
                    dc = MLP2.tile([128, 1], F32, name=f"dc{t}", tag="dc", bufs=2)
                    nc.vector.tensor_reduce(out=dc[:P, :], in_=hb[:P, :], op=ALU.add,
                                            axis=AX)
                    nc.scalar.activation(crc[t][:, :], dc[:P, :], AF.Sigmoid,
                                         bias=crb2_rep[:P, :])
                    nc.sync.dma_start(
                        ocr[TST[t]:TST[t] + P].rearrange("(p o) -> p o", o=1),
                        crc[t][:, :])

            # ---------------- S9: gumbel select + offspring + next pop ----------------
            with tc.tile_pool(name="STR", bufs=1) as STR:
                for t in range(NT):
                    P = TSZ[t]
                    sch = STR.tile([128, D], F32, name=f"sch{t}", tag="sch", bufs=2)
                    i1 = nc.sync.dma_start(sch[:P, :], spopd[TST[t]:TST[t] + P, 1:])
                    for si in scat_insts:
                        add_dep_helper(i1, si, reason="schrom2 after scatter")
                    vch = STR.tile([128, D], F32, name=f"vchs{t}", tag="vchs", bufs=2)
                    i2 = nc.sync.dma_start(vch[:P, :], vchd[TST[t]:TST[t] + P, :])
                    add_dep_helper(i2, vchd_w[t], reason="vch reload after store")
                    crs = STR.tile([128, D], F32, name=f"crs{t}", tag="crs", bufs=2)
                    nc.sync.dma_start(crs[:P, :], crossr[TST[t]:TST[t] + P, :])
                    gut = STR.tile([128, 2 * D], F32, name=f"gut{t}", tag="gut", bufs=2)
                    nc.sync.dma_start(gut[:P, :], gub[TST[t]:TST[t] + P, :])
                    et = STR.tile([128, D], F32, name=f"et{t}", tag="et", bufs=2)
                    nc.scalar.activation(et[:P, :], crs[:P, :], AF.Exp,
                                         bias=crc[t][:, 0:1], scale=-1.0)
                    l0 = STR.tile([128, D], F32, name=f"l0{t}", tag="l0", bufs=2)
                    gview = gut[:P, :].rearrange("p (f two) -> p f two", two=2)
                    nc.scalar.activation(l0[:P, :], gview[:, :, 0], AF.Ln,
                                         bias=eps10[:P, :])
                    l1 = STR.tile([128, D], F32, name=f"l1{t}", tag="l1", bufs=2)
                    nc.scalar.activation(l1[:P, :], gview[:, :, 1], AF.Ln,
                                         bias=eps10[:P, :])
                    pmul = STR.tile([128, D], F32, name=f"pm{t}", tag="pm", bufs=1)
                    nc.vector.tensor_tensor(out=pmul[:P, :], in0=l1[:P, :],
                                            in1=et[:P, :], op=ALU.mult)
                    tv = STR.tile([128, D], F32, name=f"tv{t}", tag="tv", bufs=1)
                    nc.vector.tensor_tensor(out=tv[:P, :], in0=l0[:P, :],
                                            in1=pmul[:P, :], op=ALU.subtract)
                    ym = STR.tile([128, D], U8, name=f"ym{t}", tag="ym", bufs=2)
                    nc.gpsimd.tensor_scalar(out=ym[:P, :], in0=tv[:P, :], scalar1=0.0,
                                            scalar2=None, op0=ALU.is_ge)
                    nc.vector.copy_predicated(vch[:P, :], ym[:P, :], sch[:P, :])
                    fo = STR.tile([128, 1], F32, name=f"fo{t}", tag="fo", bufs=2)
                    nc.scalar.activation(et[:P, :], vch[:P, :], AF.Square,
                                         accum_out=fo[:P, 0:1])
                    sel = STR.tile([128, 1], U8, name=f"sel{t}", tag="sel", bufs=2)
                    nc.vector.tensor_scalar(out=sel[:P, :], in0=fo[:P, :],
                                            scalar1=sfitc[t][:, 0:1], scalar2=None,
                                            op0=ALU.is_lt)
                    nc.vector.copy_predicated(sch[:P, :],
                                              sel[:P, 0:1].to_broadcast([P, D]),
                                              vch[:P, :])
                    nc.vector.copy_predicated(sfitc[t][:, :], sel[:P, 0:1], fo[:P, :])
                    nc.sync.dma_start(onext[TST[t]:TST[t] + P, 1:], sch[:P, :])
                    nc.sync.dma_start(onext[TST[t]:TST[t] + P, 0:1], sfitc[t][:, :])

    nc.compile()
    return nc


def _get_nc():
    if 'nc' not in _cache:
        _cache['nc'] = _build()
    return _cache['nc']


def _prep_rtok():
    r = np.arange(N, dtype=np.float32)
    r = (r - r.mean(dtype=np.float32)) / np.std(r, ddof=1).astype(np.float32)
    return r.astype(np.float32)


def make_in_maps(batchPop, w_w, w_b, ln1_g, ln1_b, fq_w, fq_b, fk_w, fk_b,
                 cr_w1, cr_b1, cr_ln_g, cr_ln_b, cr_w2, cr_b2,
                 mut_rand, cross_rand, gumbel_u):
    f32 = np.float32
    shared = {
        "wwT": np.ascontiguousarray(np.asarray(w_w, f32).T),
        "wb": np.asarray(w_b, f32),
        "g1": np.asarray(ln1_g, f32),
        "b1": np.asarray(ln1_b, f32),
        "fqwT": np.ascontiguousarray(np.asarray(fq_w, f32).T),
        "fqb": np.asarray(fq_b, f32),
        "fkwT": np.ascontiguousarray(np.asarray(fk_w, f32).T),
        "fkb": np.asarray(fk_b, f32),
        "crw1T": np.ascontiguousarray(np.asarray(cr_w1, f32).T),
        "crb1": np.asarray(cr_b1, f32),
        "crg": np.asarray(cr_ln_g, f32),
        "crb": np.asarray(cr_ln_b, f32),
        "crw2": np.ascontiguousarray(np.asarray(cr_w2, f32)[0]),
        "crb2": np.asarray(cr_b2, f32),
        "rtok": _prep_rtok(),
    }
    bp = np.asarray(batchPop, f32)
    mr = np.asarray(mut_rand, f32)
    cr_ = np.asarray(cross_rand, f32)
    gu = np.asarray(gumbel_u, f32).reshape(B, N, 2 * D)
    in_maps = []
    for c in range(N_CORES):
        m = dict(shared)
        m["pop"] = np.ascontiguousarray(bp[c])
        m["mutr"] = np.ascontiguousarray(mr[c])
        m["crossr"] = np.ascontiguousarray(cr_[c])
        m["gub"] = np.ascontiguousarray(gu[c])
        in_maps.append(m)
    return in_maps


def kernel(**inputs):
    from concourse import bass_utils
    nc = _get_nc()
    in_maps = make_in_maps(**inputs)
    res = bass_utils.run_bass_kernel_spmd(nc, in_maps, core_ids=list(range(N_CORES)))
    nextPop = np.stack([res.results[c]["onext"] for c in range(N_CORES)])
    A = np.stack([res.results[c]["oA"] for c in range(N_CORES)])
    paramcr = np.stack([res.results[c]["ocr"] for c in range(N_CORES)])
    return nextPop, A, paramcr


# revision 11
# speedup vs baseline: 1.0062x; 1.0062x over previous
"""Trainium2 Bass kernel for nn_GB_GLHF_1288490189083.

Data-parallel over batch: each of the 8 NeuronCores processes one population
(b=1 slice). All model weights are replicated; batchPop / mut_rand /
cross_rand / gumbel_u are sharded on dim 0.

Per-core pipeline:
  comparison-count ranks + indirect-DMA scatter sort -> fitness token
  -> tiny MLP -> q/k projections (fp32r matmuls) -> A = tanh(q k^T/sqrt(QK))
  masked, and its transpose via swapped matmul operands
  -> vchrom = A @ chrom (fp32r) -> cosine-sim token -> crossover-rate MLP
  -> gumbel-softmax hard selection -> offspring fitness + 1-to-1 selection.
"""
import sys
if '/opt/trn_rl_repo' not in sys.path:
    sys.path.insert(0, '/opt/trn_rl_repo')
import numpy as np

B, N, D = 8, 1000, 2000
H2, QK, CRH = 100, 1000, 100
NT = 8
TSZ = [128] * 7 + [104]
TST = [128 * t for t in range(NT)]
N_CORES = 8

_cache = {}


def _build():
    import concourse.bacc as bacc
    import concourse.bass as bass
    import concourse.mybir as mybir
    import concourse.tile as tile
    from concourse.tile import add_dep_helper as _adh
    from concourse.masks import make_identity

    def add_dep_helper(a, b, reason=""):
        ai = a.ins if hasattr(a, "ins") and not isinstance(a.ins, list) else a
        bi = b.ins if hasattr(b, "ins") and not isinstance(b.ins, list) else b
        _adh(ai, bi, reason=reason)

    AF = mybir.ActivationFunctionType
    ALU = mybir.AluOpType
    F32 = mybir.dt.float32
    F32R = mybir.dt.float32r
    BF16 = mybir.dt.bfloat16
    I32 = mybir.dt.int32
    U8 = mybir.dt.uint8
    AX = mybir.AxisListType.X

    nc = bacc.Bacc("TRN2", target_bir_lowering=False, debug=False,
                   num_devices=N_CORES)

    def din(name, shape):
        return nc.dram_tensor(name, shape, F32, kind="ExternalInput").ap()

    pop = din("pop", [N, D + 1])
    mutr = din("mutr", [N, N])
    crossr = din("crossr", [N, D])
    gub = din("gub", [N, 2 * D])            # gumbel_u reshaped [N, D*2]
    wwT = din("wwT", [2, H2])
    wb = din("wb", [H2])
    g1 = din("g1", [H2])
    b1 = din("b1", [H2])
    fqwT = din("fqwT", [H2, QK])
    fqb = din("fqb", [QK])
    fkwT = din("fkwT", [H2, QK])
    fkb = din("fkb", [QK])
    crw1T = din("crw1T", [3, CRH])
    crb1 = din("crb1", [CRH])
    crg = din("crg", [CRH])
    crb = din("crb", [CRH])
    crw2 = din("crw2", [CRH])
    crb2 = din("crb2", [1])
    rtok = din("rtok", [N])

    onext = nc.dram_tensor("onext", [N, D + 1], F32, kind="ExternalOutput").ap()
    oA = nc.dram_tensor("oA", [N, N], F32, kind="ExternalOutput").ap()
    ocr = nc.dram_tensor("ocr", [N], F32, kind="ExternalOutput").ap()

    spopd = nc.dram_tensor("spopd", [N, D + 1], F32, kind="Internal").ap()
    vchd = nc.dram_tensor("vchd", [N, D], F32, kind="Internal").ap()
    rnkd = nc.dram_tensor("rnkd", [N], I32, kind="Internal").ap()
    simd_ = nc.dram_tensor("simd_", [N], F32, kind="Internal").ap()

    ASCL = float(1.0 / np.sqrt(np.float32(QK), dtype=np.float32))

    with nc.allow_non_contiguous_dma(reason="small column loads/stores"), \
         tile.TileContext(nc) as tc:
        with tc.tile_pool(name="G", bufs=1) as G:
            # ---------------- constants / weights ----------------
            ident = G.tile([128, 128], F32, name="ident")
            make_identity(nc, ident[:])
            eyebf = G.tile([128, 128], BF16, name="eyebf")
            nc.vector.tensor_copy(eyebf[:], ident[:])
            onesbf = G.tile([128, 1], BF16, name="onesbf")
            nc.vector.memset(onesbf[:], 1.0)
            eps10 = G.tile([128, 1], F32, name="eps10")
            nc.vector.memset(eps10[:], 1e-10)
            epsLN = G.tile([128, 1], F32, name="epsLN")
            nc.vector.memset(epsLN[:], 1e-5)

            wwT_s = G.tile([2, H2], F32, name="wwT_s")
            nc.sync.dma_start(wwT_s[:], wwT)
            crw1T_s = G.tile([3, CRH], F32, name="crw1T_s")
            nc.sync.dma_start(crw1T_s[:], crw1T)

            def brow(name, src, width):
                row = G.tile([1, width], F32, name=name + "_r")
                nc.sync.dma_start(row[:], src.rearrange("(o f) -> o f", o=1))
                rep = G.tile([128, width], F32, name=name + "_rep")
                nc.gpsimd.partition_broadcast(rep[:], row[:1, :])
                return rep

            wb_rep = brow("wb", wb, H2)
            g1_rep = brow("g1", g1, H2)
            b1_rep = brow("b1", b1, H2)
            crb1_rep = brow("crb1", crb1, CRH)
            crg_rep = brow("crg", crg, CRH)
            crb_rep = brow("crb", crb, CRH)
            w2_rep = brow("w2", crw2, CRH)
            crb2_rep = brow("crb2", crb2, 1)

            def col(name, src, t):
                c = G.tile([TSZ[t], 1], F32, name=name)
                nc.sync.dma_start(
                    c[:], src[TST[t]:TST[t] + TSZ[t]].rearrange("(p o) -> p o", o=1))
                return c

            fqb_c = [col(f"fqb{t}", fqb, t) for t in range(NT)]
            fkb_c = [col(f"fkb{t}", fkb, t) for t in range(NT)]

            # ---------------- S1: ranks ----------------
            fitc = []
            for t in range(NT):
                c = G.tile([TSZ[t], 1], F32, name=f"fitc{t}")
                nc.sync.dma_start(c[:], pop[TST[t]:TST[t] + TSZ[t], 0:1])
                fitc.append(c)
            fitrow = G.tile([1, N], F32, name="fitrow")
            nc.sync.dma_start(fitrow[:], pop[:, 0:1].rearrange("n o -> o n"))
            fitrep = G.tile([128, N], F32, name="fitrep")
            nc.gpsimd.partition_broadcast(fitrep[:], fitrow[:1, :])

            # unsorted mean/std1 of fit (permutation-invariant)
            fbn = G.tile([1, 12], F32, name="fbn")
            for c_ in range(2):
                nc.vector.bn_stats(fbn[:, 6 * c_:6 * (c_ + 1)],
                                   fitrow[:, 500 * c_:500 * (c_ + 1)])
            fst = G.tile([1, 2], F32, name="fst")
            nc.vector.bn_aggr(fst[:], fbn[:].rearrange("o (c s) -> o c s", s=6))
            fvar1 = G.tile([1, 1], F32, name="fvar1")
            nc.vector.tensor_scalar(out=fvar1[:], in0=fst[:, 1:2],
                                    scalar1=float(N) / float(N - 1), scalar2=None,
                                    op0=ALU.mult)
            fsd = G.tile([1, 1], F32, name="fsd")
            nc.scalar.activation(fsd[:], fvar1[:], AF.Sqrt)
            finv = G.tile([1, 1], F32, name="finv")
            nc.vector.reciprocal(finv[:], fsd[:])

            ri32 = G.tile([1, N], I32, name="ri32")
            with tc.tile_pool(name="RNK", bufs=1) as RNK, \
                 tc.tile_pool(name="PS0", bufs=1, space="PSUM") as PS0:
                ltc, eqm = [], []
                for t in range(NT):
                    P = TSZ[t]
                    lt = RNK.tile([P, N], BF16, name=f"lt{t}")
                    nc.vector.tensor_scalar(out=lt[:], in0=fitrep[:P, :],
                                            scalar1=fitc[t][:, 0:1], scalar2=None,
                                            op0=ALU.is_gt)
                    eq = RNK.tile([P, N], BF16, name=f"eq{t}")
                    nc.vector.tensor_scalar(out=eq[:], in0=fitrep[:P, :],
                                            scalar1=fitc[t][:, 0:1], scalar2=None,
                                            op0=ALU.is_equal)
                    em = RNK.tile([P, N], BF16, name=f"em{t}")
                    nc.gpsimd.affine_select(em[:], eq[:], pattern=[[1, N]],
                                            compare_op=ALU.is_gt, fill=0.0,
                                            base=-TST[t], channel_multiplier=-1)
                    ltc.append(lt)
                    eqm.append(em)
                for ch in range(2):
                    sl = slice(500 * ch, 500 * (ch + 1))
                    ps = PS0.tile([1, 500], F32, name=f"rps{ch}", tag=f"rps{ch}")
                    for t in range(NT):
                        P = TSZ[t]
                        nc.tensor.matmul(ps[:], onesbf[:P, :], ltc[t][:, sl],
                                         start=(t == 0), stop=False)
                        nc.tensor.matmul(ps[:], onesbf[:P, :], eqm[t][:, sl],
                                         start=False, stop=(t == NT - 1))
                    nc.vector.tensor_copy(ri32[:, sl], ps[:])
            w_rnk = nc.sync.dma_start(rnkd.rearrange("(o f) -> o f", o=1), ri32[:])
            rankc = []
            for t in range(NT):
                rc = G.tile([TSZ[t], 1], I32, name=f"rankc{t}")
                i = nc.sync.dma_start(
                    rc[:], rnkd[TST[t]:TST[t] + TSZ[t]].rearrange("(p o) -> p o", o=1))
                add_dep_helper(i, w_rnk, reason="rank cols after rank row write")
                rankc.append(rc)

            # ---------------- S2: scatter rows into sorted order ----------------
            scat_insts = []
            with tc.tile_pool(name="SCT", bufs=1) as SCT:
                for t in range(NT):
                    P = TSZ[t]
                    pt = SCT.tile([128, D + 1], F32, name=f"popt{t}", tag="popt", bufs=2)
                    nc.sync.dma_start(pt[:P, :], pop[TST[t]:TST[t] + TSZ[t], :])
                    si = nc.gpsimd.indirect_dma_start(
                        out=spopd,
                        out_offset=bass.IndirectOffsetOnAxis(ap=rankc[t][:, 0:1], axis=0),
                        in_=pt[:P, :], in_offset=None)
                    scat_insts.append(si)

            # ---------------- S3: sorted fit + fn token ----------------
            sfitc = []
            for t in range(NT):
                c = G.tile([TSZ[t], 1], F32, name=f"sfitc{t}")
                i = nc.sync.dma_start(c[:], spopd[TST[t]:TST[t] + TSZ[t], 0:1])
                for si in scat_insts:
                    add_dep_helper(i, si, reason="sorted fit after scatter")
                sfitc.append(c)
            sfitrow = G.tile([1, N], F32, name="sfitrow")
            i = nc.sync.dma_start(sfitrow[:], spopd[:, 0:1].rearrange("n o -> o n"))
            for si in scat_insts:
                add_dep_helper(i, si, reason="sorted fit row after scatter")

            token3T = G.tile([3, N], F32, name="token3T")
            nc.vector.tensor_scalar(out=token3T[0:1, :], in0=sfitrow[:],
                                    scalar1=fst[:, 0:1], scalar2=finv[:, 0:1],
                                    op0=ALU.subtract, op1=ALU.mult)
            nc.sync.dma_start(token3T[1:2, :], rtok.rearrange("(o f) -> o f", o=1))

            simc = [G.tile([TSZ[t], 1], F32, name=f"simc{t}") for t in range(NT)]
            crc = [G.tile([TSZ[t], 1], F32, name=f"crc{t}") for t in range(NT)]
            vchd_w = []

            with tc.tile_pool(name="ATP", bufs=1) as ATP:
                ATr = [ATP.tile([TSZ[t], N], F32R, name=f"ATr{t}") for t in range(NT)]
                with tc.tile_pool(name="QKP", bufs=1) as QKP:
                    # ---------------- S4: h layer + hT ----------------
                    hT = QKP.tile([H2, N], F32R, name="hT")
                    with tc.tile_pool(name="MLP", bufs=1) as MLP, \
                         tc.tile_pool(name="PSM", bufs=1, space="PSUM") as PSM:
                        bn2all = MLP.tile([128, 2 * NT], F32, name="bn2all")
                        nc.vector.memset(bn2all[:], 1.0)
                        hbs = []
                        for t in range(NT):
                            P = TSZ[t]
                            hp = PSM.tile([P, H2], F32, name=f"hp{t}", tag="hp", bufs=2)
                            nc.tensor.matmul(hp[:], token3T[0:2, TST[t]:TST[t] + P],
                                             wwT_s[:], start=True, stop=True)
                            hb = MLP.tile([128, H2], F32, name=f"hb{t}", tag="hb", bufs=8)
                            nc.vector.tensor_tensor(out=hb[:P, :], in0=hp[:],
                                                    in1=wb_rep[:P, :], op=ALU.add)
                            nc.vector.tensor_scalar(out=hb[:P, :], in0=hb[:P, :],
                                                    scalar1=0.0, scalar2=None,
                                                    op0=ALU.max)
                            bn6 = MLP.tile([128, 6], F32, name=f"bn6{t}", tag="bn6",
                                           bufs=2)
                            nc.vector.bn_stats(bn6[:P, :], hb[:P, :])
                            nc.vector.bn_aggr(bn2all[:P, 2 * t:2 * t + 2],
                                              bn6[:P, :].rearrange(
                                                  "p (c s) -> p c s", c=1))
                            hbs.append(hb)
                        sdall = MLP.tile([128, NT], F32, name="sdall")
                        nc.scalar.activation(
                            sdall[:], bn2all[:].rearrange(
                                "p (t two) -> p t two", two=2)[:, :, 1],
                            AF.Sqrt, bias=epsLN[:, 0:1])
                        ivall = MLP.tile([128, NT], F32, name="ivall")
                        nc.vector.reciprocal(ivall[:], sdall[:])
                        for t in range(NT):
                            P = TSZ[t]
                            hb = hbs[t]
                            nc.vector.tensor_scalar(out=hb[:P, :], in0=hb[:P, :],
                                                    scalar1=bn2all[:P, 2 * t:2 * t + 1],
                                                    scalar2=ivall[:P, t:t + 1],
                                                    op0=ALU.subtract, op1=ALU.mult)
                            nc.vector.tensor_tensor(out=hb[:P, :], in0=hb[:P, :],
                                                    in1=g1_rep[:P, :], op=ALU.mult)
                            nc.vector.tensor_tensor(out=hb[:P, :], in0=hb[:P, :],
                                                    in1=b1_rep[:P, :], op=ALU.add)
                            tp = PSM.tile([H2, P], F32, name=f"tp{t}", tag="tp", bufs=2)
                            nc.tensor.transpose(tp[:], hb[:P, :H2], ident[:P, :P])
                            nc.vector.tensor_copy(hT[:, TST[t]:TST[t] + P], tp[:])

                    # ---------------- S5: q1T / k1T ----------------
                    fqwT_s = QKP.tile([H2, QK], F32R, name="fqwT_s")
                    nc.sync.dma_start(fqwT_s[:], fqwT.bitcast(F32R))
                    fkwT_s = QKP.tile([H2, QK], F32R, name="fkwT_s")
                    nc.sync.dma_start(fkwT_s[:], fkwT.bitcast(F32R))
                    q1T = [QKP.tile([TSZ[t], QK], F32R, name=f"q1T{t}")
                           for t in range(NT)]
                    k1T = [QKP.tile([TSZ[t], QK], F32R, name=f"k1T{t}")
                           for t in range(NT)]
                    with tc.tile_pool(name="PSQ", bufs=1, space="PSUM") as PSQ:
                        for mt in range(NT):
                            P = TSZ[mt]
                            for ch in range(2):
                                sl = slice(500 * ch, 500 * (ch + 1))
                                pq = PSQ.tile([P, 500], F32, name=f"pq{mt}{ch}",
                                              tag="pq", bufs=2)
                                nc.tensor.matmul(pq[:], fqwT_s[:, TST[mt]:TST[mt] + P],
                                                 hT[:, sl], start=True, stop=True)
                                nc.scalar.activation(q1T[mt][:, sl], pq[:], AF.Tanh,
                                                     bias=fqb_c[mt][:, 0:1])
                                pk = PSQ.tile([P, 500], F32, name=f"pk{mt}{ch}",
                                              tag="pk", bufs=2)
                                nc.tensor.matmul(pk[:], fkwT_s[:, TST[mt]:TST[mt] + P],
                                                 hT[:, sl], start=True, stop=True)
                                nc.scalar.activation(k1T[mt][:, sl], pk[:], AF.Tanh,
                                                     bias=fkb_c[mt][:, 0:1])

                    # ---------------- S6: A, AT, masks ----------------
                    with tc.tile_pool(name="MSKP", bufs=1) as MSKP, \
                         tc.tile_pool(name="PSA", bufs=1, space="PSUM") as PSA:
                        maskE = [MSKP.tile([128, 1024], BF16, name=f"maskE{t}")
                                 for t in range(NT)]
                        maskT = [MSKP.tile([128, 1024], BF16, name=f"maskT{t}")
                                 for t in range(NT)]
                        for t in range(NT):
                            P = TSZ[t]
                            nc.vector.memset(maskE[t][:, N:], 0.0)
                            if P < 128:
                                nc.vector.memset(maskE[t][96:, :N], 0.0)
                            mu_ = MSKP.tile([128, N], F32, name=f"mut{t}", tag="mut",
                                            bufs=2)
                            nc.sync.dma_start(mu_[:P, :], mutr[TST[t]:TST[t] + P, :])
                            nc.vector.tensor_scalar(out=maskE[t][:P, :N], in0=mu_[:P, :],
                                                    scalar1=0.5, scalar2=None,
                                                    op0=ALU.is_ge)
                            nc.vector.tensor_tensor(
                                out=maskE[t][:P, TST[t]:TST[t] + P],
                                in0=maskE[t][:P, TST[t]:TST[t] + P],
                                in1=eyebf[:P, :P], op=ALU.max)
                        # A (natural) -> masked -> DMA out
                        for i in range(NT):
                            P = TSZ[i]
                            at_ = MSKP.tile([128, N], F32, name=f"at{i}", tag="at",
                                            bufs=2)
                            for ch in range(2):
                                sl = slice(500 * ch, 500 * (ch + 1))
                                pa = PSA.tile([P, 500], F32, name=f"pa{i}{ch}",
                                              tag="pa", bufs=2)
                                for mt in range(NT):
                                    nc.tensor.matmul(pa[:],
                                                     q1T[mt][:, TST[i]:TST[i] + P],
                                                     k1T[mt][:, sl], start=(mt == 0),
                                                     stop=(mt == NT - 1))
                                nc.scalar.activation(at_[:P, sl], pa[:], AF.Tanh,
                                                     scale=ASCL)
                            nc.vector.tensor_tensor(out=at_[:P, :N], in0=at_[:P, :N],
                                                    in1=maskE[i][:P, :N], op=ALU.mult)
                            nc.sync.dma_start(oA[TST[i]:TST[i] + P, :], at_[:P, :N])
                        # mask transpose blocks (bf16 xbar DMA transpose)
                        for t in range(NT):
                            for j in range(NT):
                                nc.sync.dma_start_transpose(
                                    maskT[j][:, 128 * t:128 * (t + 1)],
                                    maskE[t][:, 128 * j:128 * (j + 1)])
                        # AT (swapped operands) -> masked -> f32r
                        for i in range(NT):
                            P = TSZ[i]
                            att_ = MSKP.tile([128, N], F32, name=f"att{i}", tag="att",
                                             bufs=2)
                            for ch in range(2):
                                sl = slice(500 * ch, 500 * (ch + 1))
                                pb_ = PSA.tile([P, 500], F32, name=f"pb{i}{ch}",
                                               tag="pb", bufs=2)
                                for mt in range(NT):
                                    nc.tensor.matmul(pb_[:],
                                                     k1T[mt][:, TST[i]:TST[i] + P],
                                                     q1T[mt][:, sl], start=(mt == 0),
                                                     stop=(mt == NT - 1))
                                nc.scalar.activation(att_[:P, sl], pb_[:], AF.Tanh,
                                                     scale=ASCL)
                            nc.vector.tensor_tensor(out=ATr[i][:P, :], in0=att_[:P, :N],
                                                    in1=maskT[i][:P, :N], op=ALU.mult)

                # ---------------- S7: vchrom + sim stats ----------------
                with tc.tile_pool(name="SC", bufs=1) as SC, \
                     tc.tile_pool(name="VC", bufs=1) as VC, \
                     tc.tile_pool(name="PSV", bufs=1, space="PSUM") as PSV:
                    schrom = []
                    for t in range(NT):
                        P = TSZ[t]
                        s = SC.tile([P, D], F32R, name=f"schrom{t}")
                        i = nc.sync.dma_start(
                            s[:], spopd[TST[t]:TST[t] + P, 1:].bitcast(F32R))
                        for si in scat_insts:
                            add_dep_helper(i, si, reason="sorted chrom after scatter")
                        schrom.append(s)
                    denall = VC.tile([128, NT], F32, name="denall")
                    nc.vector.memset(denall[:], 1.0)
                    dts = []
                    for i in range(NT):
                        P = TSZ[i]
                        vch = VC.tile([128, D], F32, name=f"vch{i}", tag="vch", bufs=2)
                        svp = VC.tile([128, 4], F32, name=f"svp{i}", tag="svp", bufs=2)
                        for ch in range(4):
                            sl = slice(500 * ch, 500 * (ch + 1))
                            pv = PSV.tile([P, 500], F32, name=f"pv{i}{ch}", tag="pv",
                                          bufs=2)
                            for mt in range(NT):
                                nc.tensor.matmul(pv[:], ATr[mt][:, TST[i]:TST[i] + P],
                                                 schrom[mt][:, sl], start=(mt == 0),
                                                 stop=(mt == NT - 1))
                            nc.scalar.activation(vch[:P, sl], pv[:], AF.Identity,
                                                 accum_out=svp[:P, ch:ch + 1])
                        vchd_w.append(
                            nc.sync.dma_start(vchd[TST[i]:TST[i] + P, :], vch[:P, :]))
                        sv = VC.tile([128, 1], F32, name=f"sv{i}", tag="sv", bufs=2)
                        nc.vector.tensor_reduce(out=sv[:P, :], in_=svp[:P, :],
                                                op=ALU.add, axis=AX)
                        scr = VC.tile([128, D], F32, name=f"scr{i}", tag="scr", bufs=2)
                        svv = VC.tile([128, 1], F32, name=f"svv{i}", tag="svv", bufs=2)
                        nc.scalar.activation(scr[:P, :], vch[:P, :], AF.Square,
                                             accum_out=svv[:P, 0:1])
                        cbn = VC.tile([128, 24], F32, name=f"cbn{i}", tag="cbn", bufs=2)
                        for c_ in range(4):
                            nc.vector.bn_stats(
                                cbn[:P, 6 * c_:6 * (c_ + 1)],
                                schrom[i][:, 500 * c_:500 * (c_ + 1)].bitcast(F32))
                        cst = VC.tile([128, 2], F32, name=f"cst{i}", tag="cst", bufs=2)
                        nc.vector.bn_aggr(cst[:P, :],
                                          cbn[:P, :].rearrange("p (c s) -> p c s", s=6))
                        prod = VC.tile([128, D], F32, name=f"prod{i}", tag="prod",
                                       bufs=2)
                        nc.vector.tensor_tensor(out=prod[:P, :],
                                                in0=schrom[i][:].bitcast(F32),
                                                in1=vch[:P, :], op=ALU.mult)
                        scv = VC.tile([128, 1], F32, name=f"scv{i}", tag="scv", bufs=2)
                        nc.scalar.activation(scr[:P, :], prod[:P, :], AF.Identity,
                                             accum_out=scv[:P, 0:1])
                        # column math for cosine sim
                        sc_ = VC.tile([128, 1], F32, name=f"sc{i}", tag="sc_", bufs=2)
                        nc.vector.tensor_scalar(out=sc_[:P, :], in0=cst[:P, 0:1],
                                                scalar1=float(D), scalar2=None,
                                                op0=ALU.mult)
                        scc = VC.tile([128, 1], F32, name=f"scc{i}", tag="scc", bufs=2)
                        nc.vector.tensor_scalar(out=scc[:P, :], in0=cst[:P, 1:2],
                                                scalar1=float(D), scalar2=None,
                                                op0=ALU.mult)
                        msq = VC.tile([128, 1], F32, name=f"msq{i}", tag="msq", bufs=2)
                        nc.vector.tensor_scalar(out=msq[:P, :], in0=cst[:P, 0:1],
                                                scalar1=cst[:P, 0:1], scalar2=float(D),
                                                op0=ALU.mult, op1=ALU.mult)
                        nc.vector.tensor_tensor(out=scc[:P, :], in0=scc[:P, :],
                                                in1=msq[:P, :], op=ALU.add)
                        mrow = VC.tile([128, 1], F32, name=f"mrow{i}", tag="mrow",
                                       bufs=2)
                        nc.vector.tensor_tensor(out=mrow[:P, :], in0=sc_[:P, :],
                                                in1=sv[:P, :], op=ALU.add)
                        nc.vector.tensor_scalar(out=mrow[:P, :], in0=mrow[:P, :],
                                                scalar1=1.0 / (2.0 * D), scalar2=None,
                                                op0=ALU.mult)
                        m2d = VC.tile([128, 1], F32, name=f"m2d{i}", tag="m2d", bufs=2)
                        nc.vector.tensor_scalar(out=m2d[:P, :], in0=mrow[:P, :],
                                                scalar1=mrow[:P, 0:1], scalar2=float(D),
                                                op0=ALU.mult, op1=ALU.mult)
                        t1 = VC.tile([128, 1], F32, name=f"t1{i}", tag="t1", bufs=2)
                        nc.vector.tensor_scalar(out=t1[:P, :], in0=sc_[:P, :],
                                                scalar1=mrow[:P, 0:1], scalar2=-2.0,
                                                op0=ALU.mult, op1=ALU.mult)
                        nc2_ = VC.tile([128, 1], F32, name=f"nc2{i}", tag="nc2", bufs=2)
                        nc.vector.tensor_tensor(out=nc2_[:P, :], in0=scc[:P, :],
                                                in1=t1[:P, :], op=ALU.add)
                        nc.vector.tensor_tensor(out=nc2_[:P, :], in0=nc2_[:P, :],
                                                in1=m2d[:P, :], op=ALU.add)
                        t2 = VC.tile([128, 1], F32, name=f"t2{i}", tag="t2", bufs=2)
                        nc.vector.tensor_scalar(out=t2[:P, :], in0=sv[:P, :],
                                                scalar1=mrow[:P, 0:1], scalar2=-2.0,
                                                op0=ALU.mult, op1=ALU.mult)
                        nv2_ = VC.tile([128, 1], F32, name=f"nv2{i}", tag="nv2", bufs=2)
                        nc.vector.tensor_tensor(out=nv2_[:P, :], in0=svv[:P, :],
                                                in1=t2[:P, :], op=ALU.add)
                        nc.vector.tensor_tensor(out=nv2_[:P, :], in0=nv2_[:P, :],
                                                in1=m2d[:P, :], op=ALU.add)
                        dt_ = VC.tile([128, 1], F32, name=f"dt{i}", tag="dt", bufs=8)
                        nc.vector.tensor_tensor(out=dt_[:P, :], in0=sc_[:P, :],
                                                in1=sv[:P, :], op=ALU.add)
                        nc.vector.tensor_scalar(out=dt_[:P, :], in0=dt_[:P, :],
                                                scalar1=mrow[:P, 0:1], scalar2=-1.0,
                                                op0=ALU.mult, op1=ALU.mult)
                        nc.vector.tensor_tensor(out=dt_[:P, :], in0=dt_[:P, :],
                                                in1=scv[:P, :], op=ALU.add)
                        nc.vector.tensor_tensor(out=dt_[:P, :], in0=dt_[:P, :],
                                                in1=m2d[:P, :], op=ALU.add)
                        nc.vector.tensor_tensor(out=denall[:P, i:i + 1],
                                                in0=nc2_[:P, :],
                                                in1=nv2_[:P, :], op=ALU.mult)
                        dts.append(dt_)

                    sdden = VC.tile([128, NT], F32, name="sdden")
                    nc.scalar.activation(sdden[:], denall[:], AF.Sqrt)
                    nc.vector.reciprocal(sdden[:], sdden[:])
                    for i in range(NT):
                        P = TSZ[i]
                        nc.vector.tensor_tensor(out=simc[i][:, :], in0=dts[i][:P, :],
                                                in1=sdden[:P, i:i + 1], op=ALU.mult)

            # ---------------- S8: sim normalize + cr MLP ----------------
            sim_w = []
            for t in range(NT):
                sim_w.append(nc.sync.dma_start(
                    simd_[TST[t]:TST[t] + TSZ[t]].rearrange("(p o) -> p o", o=1),
                    simc[t][:, :]))
            simrow = G.tile([1, N], F32, name="simrow")
            i = nc.sync.dma_start(simrow[:], simd_.rearrange("(o f) -> o f", o=1))
            for w in sim_w:
                add_dep_helper(i, w, reason="sim row after col writes")
            sbn = G.tile([1, 12], F32, name="sbn")
            for c_ in range(2):
                nc.vector.bn_stats(sbn[:, 6 * c_:6 * (c_ + 1)],
                                   simrow[:, 500 * c_:500 * (c_ + 1)])
            sst = G.tile([1, 2], F32, name="sst")
            nc.vector.bn_aggr(sst[:], sbn[:].rearrange("o (c s) -> o c s", s=6))
            svar1 = G.tile([1, 1], F32, name="svar1")
            nc.vector.tensor_scalar(out=svar1[:], in0=sst[:, 1:2],
                                    scalar1=float(N) / float(N - 1), scalar2=None,
                                    op0=ALU.mult)
            ssd = G.tile([1, 1], F32, name="ssd")
            nc.scalar.activation(ssd[:], svar1[:], AF.Sqrt)
            sinv = G.tile([1, 1], F32, name="sinv")
            nc.vector.reciprocal(sinv[:], ssd[:])
            simn = G.tile([1, N], F32, name="simn")
            nc.vector.tensor_scalar(out=simn[:], in0=simrow[:],
                                    scalar1=sst[:, 0:1], scalar2=sinv[:, 0:1],
                                    op0=ALU.subtract, op1=ALU.mult)
            nc.sync.dma_start(token3T[2:3, :], simn[:])

            with tc.tile_pool(name="MLP2", bufs=1) as MLP2, \
                 tc.tile_pool(name="PSM2", bufs=1, space="PSUM") as PSM2:
                cb2all = MLP2.tile([128, 2 * NT], F32, name="cb2all")
                nc.vector.memset(cb2all[:], 1.0)
                h2s = []
                for t in range(NT):
                    P = TSZ[t]
                    hp = PSM2.tile([P, CRH], F32, name=f"h2p{t}", tag="h2p", bufs=2)
                    nc.tensor.matmul(hp[:], token3T[:, TST[t]:TST[t] + P],
                                     crw1T_s[:], start=True, stop=True)
                    hb = MLP2.tile([128, CRH], F32, name=f"h2b{t}", tag="h2b", bufs=8)
                    nc.vector.tensor_tensor(out=hb[:P, :], in0=hp[:],
                                            in1=crb1_rep[:P, :], op=ALU.add)
                    nc.vector.tensor_scalar(out=hb[:P, :], in0=hb[:P, :], scalar1=0.0,
                                            scalar2=None, op0=ALU.max)
                    bn6 = MLP2.tile([128, 6], F32, name=f"c6{t}", tag="c6", bufs=2)
                    nc.vector.bn_stats(bn6[:P, :], hb[:P, :])
                    nc.vector.bn_aggr(cb2all[:P, 2 * t:2 * t + 2],
                                      bn6[:P, :].rearrange("p (c s) -> p c s", c=1))
                    h2s.append(hb)
                csdall = MLP2.tile([128, NT], F32, name="csdall")
                nc.scalar.activation(
                    csdall[:], cb2all[:].rearrange(
                        "p (t two) -> p t two", two=2)[:, :, 1],
                    AF.Sqrt, bias=epsLN[:, 0:1])
                civall = MLP2.tile([128, NT], F32, name="civall")
                nc.vector.reciprocal(civall[:], csdall[:])
                dcall = MLP2.tile([128, NT], F32, name="dcall")
                nc.vector.memset(dcall[:], 0.0)
                for t in range(NT):
                    P = TSZ[t]
                    hb = h2s[t]
                    nc.vector.tensor_scalar(out=hb[:P, :], in0=hb[:P, :],
                                            scalar1=cb2all[:P, 2 * t:2 * t + 1],
                                            scalar2=civall[:P, t:t + 1],
                                            op0=ALU.subtract, op1=ALU.mult)
                    nc.vector.tensor_tensor(out=hb[:P, :], in0=hb[:P, :],
                                            in1=crg_rep[:P, :], op=ALU.mult)
                    nc.vector.tensor_tensor(out=hb[:P, :], in0=hb[:P, :],
                                            in1=crb_rep[:P, :], op=ALU.add)
                    nc.vector.tensor_tensor(out=hb[:P, :], in0=hb[:P, :],
                                            in1=w2_rep[:P, :], op=ALU.mult)
                    nc.vector.tensor_reduce(out=dcall[:P, t:t + 1], in_=hb[:P, :],
                                            op=ALU.add, axis=AX)
                crall = G.tile([128, NT], F32, name="crall")
                nc.scalar.activation(crall[:], dcall[:], AF.Sigmoid,
                                     bias=crb2_rep[:, 0:1])
                for t in range(NT):
                    P = TSZ[t]
                    nc.vector.tensor_copy(crc[t][:, :], crall[:P, t:t + 1])
                    nc.sync.dma_start(
                        ocr[TST[t]:TST[t] + P].rearrange("(p o) -> p o", o=1),
                        crc[t][:, :])

            # ---------------- S9: gumbel select + offspring + next pop ----------------
            with tc.tile_pool(name="STR", bufs=1) as STR:
                for t in range(NT):
                    P = TSZ[t]
                    sch = STR.tile([128, D], F32, name=f"sch{t}", tag="sch", bufs=2)
                    i1 = nc.sync.dma_start(sch[:P, :], spopd[TST[t]:TST[t] + P, 1:])
                    for si in scat_insts:
                        add_dep_helper(i1, si, reason="schrom2 after scatter")
                    vch = STR.tile([128, D], F32, name=f"vchs{t}", tag="vchs", bufs=2)
                    i2 = nc.sync.dma_start(vch[:P, :], vchd[TST[t]:TST[t] + P, :])
                    add_dep_helper(i2, vchd_w[t], reason="vch reload after store")
                    crs = STR.tile([128, D], F32, name=f"crs{t}", tag="crs", bufs=2)
                    nc.sync.dma_start(crs[:P, :], crossr[TST[t]:TST[t] + P, :])
                    gut = STR.tile([128, 2 * D], F32, name=f"gut{t}", tag="gut", bufs=2)
                    nc.sync.dma_start(gut[:P, :], gub[TST[t]:TST[t] + P, :])
                    # all-Ln gumbel compare: pick chrom iff
                    #   cr + g0 >= cross + g1  <=>  M1 - M0 >= cross - cr
                    # with Mi = ln(1e-10 - ln(u_i + 1e-10))
                    cc = STR.tile([128, D], F32, name=f"cc{t}", tag="cc", bufs=1)
                    nc.gpsimd.tensor_scalar(out=cc[:P, :], in0=crs[:P, :],
                                            scalar1=crc[t][:, 0:1], scalar2=None,
                                            op0=ALU.subtract)
                    l0 = STR.tile([128, D], F32, name=f"l0{t}", tag="l0", bufs=2)
                    gview = gut[:P, :].rearrange("p (f two) -> p f two", two=2)
                    nc.scalar.activation(l0[:P, :], gview[:, :, 0], AF.Ln,
                                         bias=eps10[:P, :])
                    l1 = STR.tile([128, D], F32, name=f"l1{t}", tag="l1", bufs=2)
                    nc.scalar.activation(l1[:P, :], gview[:, :, 1], AF.Ln,
                                         bias=eps10[:P, :])
                    m0 = STR.tile([128, D], F32, name=f"m0{t}", tag="et", bufs=2)
                    nc.scalar.activation(m0[:P, :], l0[:P, :], AF.Ln,
                                         bias=eps10[:P, :], scale=-1.0)
                    m1 = STR.tile([128, D], F32, name=f"m1{t}", tag="tv", bufs=2)
                    nc.scalar.activation(m1[:P, :], l1[:P, :], AF.Ln,
                                         bias=eps10[:P, :], scale=-1.0)
                    d1 = STR.tile([128, D], F32, name=f"d1{t}", tag="pm", bufs=1)
                    nc.vector.tensor_tensor(out=d1[:P, :], in0=m1[:P, :],
                                            in1=m0[:P, :], op=ALU.subtract)
                    ym = STR.tile([128, D], U8, name=f"ym{t}", tag="ym", bufs=2)
                    nc.vector.tensor_tensor(out=ym[:P, :], in0=d1[:P, :],
                                            in1=cc[:P, :], op=ALU.is_ge)
                    nc.vector.copy_predicated(vch[:P, :], ym[:P, :], sch[:P, :])
                    fo = STR.tile([128, 1], F32, name=f"fo{t}", tag="fo", bufs=2)
                    sqscr = STR.tile([128, D], F32, name=f"sq{t}", tag="cc", bufs=1)
                    nc.scalar.activation(sqscr[:P, :], vch[:P, :], AF.Square,
                                         accum_out=fo[:P, 0:1])
                    sel = STR.tile([128, 1], U8, name=f"sel{t}", tag="sel", bufs=2)
                    nc.vector.tensor_scalar(out=sel[:P, :], in0=fo[:P, :],
                                            scalar1=sfitc[t][:, 0:1], scalar2=None,
                                            op0=ALU.is_lt)
                    nc.vector.copy_predicated(sch[:P, :],
                                              sel[:P, 0:1].to_broadcast([P, D]),
                                              vch[:P, :])
                    nc.vector.copy_predicated(sfitc[t][:, :], sel[:P, 0:1], fo[:P, :])
                    nc.sync.dma_start(onext[TST[t]:TST[t] + P, 1:], sch[:P, :])
                    nc.sync.dma_start(onext[TST[t]:TST[t] + P, 0:1], sfitc[t][:, :])

    nc.compile()
    return nc


def _get_nc():
    if 'nc' not in _cache:
        _cache['nc'] = _build()
    return _cache['nc']


def _prep_rtok():
    r = np.arange(N, dtype=np.float32)
    r = (r - r.mean(dtype=np.float32)) / np.std(r, ddof=1).astype(np.float32)
    return r.astype(np.float32)


def make_in_maps(batchPop, w_w, w_b, ln1_g, ln1_b, fq_w, fq_b, fk_w, fk_b,
                 cr_w1, cr_b1, cr_ln_g, cr_ln_b, cr_w2, cr_b2,
                 mut_rand, cross_rand, gumbel_u):
    f32 = np.float32
    shared = {
        "wwT": np.ascontiguousarray(np.asarray(w_w, f32).T),
        "wb": np.asarray(w_b, f32),
        "g1": np.asarray(ln1_g, f32),
        "b1": np.asarray(ln1_b, f32),
        "fqwT": np.ascontiguousarray(np.asarray(fq_w, f32).T),
        "fqb": np.asarray(fq_b, f32),
        "fkwT": np.ascontiguousarray(np.asarray(fk_w, f32).T),
        "fkb": np.asarray(fk_b, f32),
        "crw1T": np.ascontiguousarray(np.asarray(cr_w1, f32).T),
        "crb1": np.asarray(cr_b1, f32),
        "crg": np.asarray(cr_ln_g, f32),
        "crb": np.asarray(cr_ln_b, f32),
        "crw2": np.ascontiguousarray(np.asarray(cr_w2, f32)[0]),
        "crb2": np.asarray(cr_b2, f32),
        "rtok": _prep_rtok(),
    }
    bp = np.asarray(batchPop, f32)
    mr = np.asarray(mut_rand, f32)
    cr_ = np.asarray(cross_rand, f32)
    gu = np.asarray(gumbel_u, f32).reshape(B, N, 2 * D)
    in_maps = []
    for c in range(N_CORES):
        m = dict(shared)
        m["pop"] = np.ascontiguousarray(bp[c])
        m["mutr"] = np.ascontiguousarray(mr[c])
        m["crossr"] = np.ascontiguousarray(cr_[c])
        m["gub"] = np.ascontiguousarray(gu[c])
        in_maps.append(m)
    return in_maps


def kernel(**inputs):
    from concourse import bass_utils
    nc = _get_nc()
    in_maps = make_in_maps(**inputs)
    res = bass_utils.run_bass_kernel_spmd(nc, in_maps, core_ids=list(range(N_CORES)))
    nextPop = np.stack([res.results[c]["onext"] for c in range(N_CORES)])
    A = np.stack([res.results[c]["oA"] for c in range(N_CORES)])
    paramcr = np.stack([res.results[c]["ocr"] for c in range(N_CORES)])
    return nextPop, A, paramcr
